# revision 25
# baseline (speedup 1.0000x reference)
"""Bass/Trainium2 kernel for nn_EDA_Attention (CSWin-style strip attention +
dynamic depthwise-conv positional branch).

Contract: kernel(**inputs) takes the FULL unsharded inputs (B=16) and returns
the FULL output (16, 2304, 128) float32. Internally shards data-parallel over
B across 8 NeuronCores (2 samples per core), runs one SPMD Bass program via
run_bass_kernel_spmd, and concatenates the per-core outputs.
"""

import os
from contextlib import ExitStack

import numpy as np

import concourse.bass as bass
import concourse.mybir as mybir
import concourse.tile as tile
from concourse.masks import make_identity

F32 = mybir.dt.float32

# The pinned walrus rejects instructions carrying more than one semaphore
# wait ("Too many sync wait commands"). After Tile scheduling, hoist excess
# waits onto same-engine NOPs inserted immediately before the instruction.
# Safe: each engine queue executes in order, and Tile only emits waits on
# semaphores whose increments were already enqueued.
def _split_sync_waits(nc, max_waits=1):
    counter = [0]
    for func in nc.m.functions:
        for blk in func.blocks:
            new_insts = []
            changed = False
            for inst in blk.instructions:
                si = inst.sync_info
                if si is not None and len(si.on_wait) > max_waits:
                    waits = list(si.on_wait)
                    keep, extra = waits[-max_waits:], waits[:-max_waits]
                    for w in extra:
                        counter[0] += 1
                        nop = mybir.InstNoOp(
                            name=f"I-wsplit-{counter[0]}", ins=[], outs=[]
                        )
                        nop.engine = inst.engine
                        nop.sync_info = mybir.SyncInfo(on_wait=[w], on_update=[])
                        nc.register_instruction(nop, overwrite=True)
                        new_insts.append(nop)
                    inst.sync_info = mybir.SyncInfo(
                        on_wait=keep, on_update=list(si.on_update)
                    )
                    changed = True
                new_insts.append(inst)
            if changed:
                blk.instructions = new_insts
F32R = mybir.dt.float32r
BF16 = mybir.dt.bfloat16
AF = mybir.ActivationFunctionType
ALU = mybir.AluOpType

# Problem constants (hardcoded per spec).
B_FULL = 16
N_CORES = 8
BS = B_FULL // N_CORES  # samples per core
H = 48
W = 48
C = 128
L = H * W               # 2304
G = 4
NH = 8
HD = C // NH            # 16
SS = 8                  # strip (window) width; idx=0 -> vertical strips
NWIN = W // SS          # 6 windows per sample
LW = H * SS             # 384 tokens per window
NCH = LW // 128         # 3 chunks of 128 tokens
KK = 3                  # conv kernel size
SCALE = HD ** -0.5      # attention scale (0.25)
GELU_A = 0.7978845608028654  # sqrt(2/pi)
GELU_B = 0.044715

P = 128


def _emit(ctx: ExitStack, tc, q, k, v, p1wT, eb1, p2wT, p2br, dwb, out):
    nc = tc.nc

    singles = ctx.enter_context(tc.tile_pool(name="singles", bufs=1))
    samp = ctx.enter_context(tc.tile_pool(name="samp", bufs=2))
    win = ctx.enter_context(tc.tile_pool(name="win", bufs=2))
    xnp = ctx.enter_context(tc.tile_pool(name="xnp", bufs=4))
    smalls = ctx.enter_context(tc.tile_pool(name="smalls", bufs=2))
    sp_pool = ctx.enter_context(tc.tile_pool(name="sp", bufs=1, space="PSUM"))
    up_pool = ctx.enter_context(tc.tile_pool(name="up", bufs=2, space="PSUM"))
    scr_pool = ctx.enter_context(tc.tile_pool(name="scr", bufs=2, space="PSUM"))

    ident = singles.tile([P, P], F32)
    make_identity(nc, ident)
    identb = singles.tile([P, P], BF16, tag="identb")
    nc.vector.tensor_copy(identb, ident)
    p1wT_sb = singles.tile([P, C // 4], F32, tag="p1wT")
    nc.sync.dma_start(p1wT_sb, p1wT)
    eb1_sb = singles.tile([C // 4, 1], F32, tag="eb1")
    nc.sync.dma_start(eb1_sb, eb1)
    p2wT_sb = singles.tile([C // 4, G * C], F32, tag="p2wT")
    nc.sync.dma_start(p2wT_sb, p2wT)
    p2br_sb = singles.tile([P, G], F32, tag="p2br")
    nc.sync.dma_start(p2br_sb, p2br)
    dwb_sb = singles.tile([P, G, 10], F32, tag="dwb")
    nc.sync.dma_start(dwb_sb, dwb)

    for s in range(BS):
        q3 = q[s].rearrange("(h w) c -> h w c", w=W)
        k3 = k[s].rearrange("(h w) c -> h w c", w=W)
        v3 = v[s].rearrange("(h w) c -> h w c", w=W)
        o3 = out[s].rearrange("(h w) c -> h w c", w=W)

        # ---------- sample prologue: v in natural + image layout ----------
        vn = samp.tile([P, NWIN * NCH, P], F32, tag="vn")
        for w in range(NWIN):
            for cc in range(NCH):
                nc.sync.dma_start(
                    vn[:, w * NCH + cc, :],
                    v3[16 * cc : 16 * cc + 16, SS * w : SS * w + SS, :],
                )

        # zero-padded image [C, H+2, W+2] for the 3x3 conv (bf16)
        vnb = samp.tile([P, NWIN * NCH, P], BF16, tag="vnb")
        nc.vector.tensor_copy(vnb, vn)
        vpad = samp.tile([P, H + 2, W + 2], BF16, tag="vpad")
        nc.vector.memset(vpad[:, 0, :], 0.0)
        nc.vector.memset(vpad[:, H + 1, :], 0.0)
        nc.vector.memset(vpad[:, 1 : H + 1, 0:1], 0.0)
        nc.vector.memset(vpad[:, 1 : H + 1, W + 1 : W + 2], 0.0)
        for w in range(NWIN):
            for cc in range(NCH):
                tp = scr_pool.tile([P, 480], BF16, tag="scr")
                nc.tensor.transpose(tp[:, 0:P], vnb[:, w * NCH + cc, :], identb)
                nc.vector.tensor_copy(
                    vpad[:, 1 + 16 * cc : 1 + 16 * cc + 16, 1 + SS * w : 1 + SS * w + SS],
                    tp[:, 0:P].rearrange("p (a b) -> p a b", b=SS),
                )

        # ---------- dynamic conv weight branch ----------
        # pooled 3x3 block sums (AdaptiveAvgPool to 3x3 -> / 256)
        pooled = smalls.tile([P, KK, KK], F32, tag="pooled")
        nc.vector.tensor_reduce(
            out=pooled,
            in_=vpad[:, 1 : H + 1, 1 : W + 1].rearrange(
                "p (a y) (b x) -> p a b y x", y=16, x=16
            ),
            axis=mybir.AxisListType.XY,
            op=ALU.add,
        )
        rhs10 = smalls.tile([P, 10], F32, tag="rhs10")
        nc.vector.tensor_scalar(
            out=rhs10[:, 0:9],
            in0=pooled.rearrange("p a b -> p (a b)"),
            scalar1=1.0 / 256.0,
            scalar2=None,
            op0=ALU.mult,
        )
        gsum = smalls.tile([P, 1], F32, tag="gsum")
        nc.vector.tensor_reduce(
            out=gsum, in_=pooled, axis=mybir.AxisListType.XY, op=ALU.add
        )
        nc.vector.tensor_scalar(
            out=rhs10[:, 9:10], in0=gsum, scalar1=1.0 / float(L), scalar2=None,
            op0=ALU.mult,
        )
        # proj1 (BN folded host-side) -> gelu(tanh approx; x0.5 folded into p2w)
        pj1 = scr_pool.tile([P, 480], F32, tag="scr")
        nc.tensor.matmul(pj1[0 : C // 4, 0:10], p1wT_sb, rhs10, start=True, stop=True)
        g0 = smalls.tile([C // 4, 10], F32, tag="g0")
        nc.vector.tensor_scalar(
            out=g0, in0=pj1[0 : C // 4, 0:10], scalar1=eb1_sb, scalar2=None,
            op0=ALU.add,
        )
        x2 = smalls.tile([C // 4, 10], F32, tag="x2")
        nc.vector.tensor_mul(x2, g0, g0)
        x3 = smalls.tile([C // 4, 10], F32, tag="x3")
        nc.vector.tensor_mul(x3, x2, g0)
        inner = smalls.tile([C // 4, 10], F32, tag="inner")
        nc.vector.scalar_tensor_tensor(
            out=inner, in0=x3, scalar=GELU_B, in1=g0, op0=ALU.mult, op1=ALU.add
        )
        th = smalls.tile([C // 4, 10], F32, tag="th")
        nc.scalar.activation(th, inner, AF.Tanh, scale=GELU_A)
        ge = smalls.tile([C // 4, 10], F32, tag="ge")
        nc.vector.scalar_tensor_tensor(
            out=ge, in0=th, scalar=1.0, in1=g0, op0=ALU.add, op1=ALU.mult
        )
        # proj2 per group, + bias
        s_sb = smalls.tile([P, G, 10], F32, tag="s_sb")
        for g in range(G):
            pj2 = scr_pool.tile([P, 480], F32, tag="scr")
            nc.tensor.matmul(
                pj2[:, 0:10], p2wT_sb[:, P * g : P * (g + 1)], ge,
                start=True, stop=True,
            )
            nc.vector.tensor_scalar(
                out=s_sb[:, g, :], in0=pj2[:, 0:10], scalar1=p2br_sb[:, g : g + 1],
                scalar2=None, op0=ALU.add,
            )
        # softmax over G (no max subtraction; values are small) + weighted sum
        e_sb = smalls.tile([P, G, 10], F32, tag="e_sb")
        nc.scalar.activation(e_sb, s_sb, AF.Exp)
        esum = smalls.tile([P, 10], F32, tag="esum")
        nc.vector.tensor_add(esum, e_sb[:, 0, :], e_sb[:, 1, :])
        nc.vector.tensor_add(esum, esum, e_sb[:, 2, :])
        nc.vector.tensor_add(esum, esum, e_sb[:, 3, :])
        rs = smalls.tile([P, 10], F32, tag="rs")
        nc.vector.reciprocal(rs, esum)
        wnum = smalls.tile([P, 10], F32, tag="wnum")
        tmpg = smalls.tile([P, 10], F32, tag="tmpg")
        nc.vector.tensor_mul(wnum, e_sb[:, 0, :], dwb_sb[:, 0, :])
        for g in range(1, G):
            nc.vector.tensor_mul(tmpg, e_sb[:, g, :], dwb_sb[:, g, :])
            nc.vector.tensor_add(wnum, wnum, tmpg)
        wb = smalls.tile([P, 10], F32, tag="wb")  # cols 0:9 taps, col 9 bias
        nc.vector.tensor_mul(wb, wnum, rs)
        # diagonal weight tiles for the depthwise conv-as-matmul
        diag9 = samp.tile([P, 9, P], BF16, tag="diag9")
        for t in range(9):
            nc.vector.tensor_scalar(
                out=diag9[:, t, :], in0=ident, scalar1=wb[:, t : t + 1],
                scalar2=None, op0=ALU.mult,
            )
        # depthwise 3x3 conv: 9 shifted diag-matmuls accumulated in PSUM.
        # Output written straight into window-chunk-major layout so the
        # pos transposes below read contiguous [128, 128] slices.
        pos_wcm = samp.tile([P, NWIN * NCH, P], BF16, tag="pos_wcm")
        povw = pos_wcm.rearrange(
            "p (w c2) (h2 r s) -> p w c2 h2 r s", c2=NCH, r=8, s=SS
        )
        for cc in range(NCH):
            for hf in range(2):
                r0 = 16 * cc + 8 * hf
                cp = scr_pool.tile([P, 512], F32, tag="scr")
                for t in range(9):
                    dy, dx = t // 3 - 1, t % 3 - 1
                    nc.tensor.matmul(
                        cp[:, 0 : 8 * W],
                        diag9[:, t, :],
                        vpad[:, 1 + r0 + dy : 1 + r0 + dy + 8, 1 + dx : 1 + dx + W],
                        start=(t == 0),
                        stop=(t == 8),
                    )
                nc.vector.tensor_scalar(
                    out=povw[:, :, cc, hf, :, :].rearrange("p w r s -> p r w s"),
                    in0=cp[:, 0 : 8 * W].rearrange("p (r w s) -> p r w s", w=NWIN, s=SS),
                    scalar1=wb[:, 9:10],
                    scalar2=None,
                    op0=ALU.add,
                )
        # transpose pos back to [token, C] per window chunk
        post = samp.tile([P, NWIN * NCH, P], F32, tag="post")
        for w in range(NWIN):
            for cc in range(NCH):
                tp = scr_pool.tile([P, 480], BF16, tag="scr")
                nc.tensor.transpose(
                    tp[:, 0:P],
                    pos_wcm[:, w * NCH + cc, :],
                    identb,
                )
                nc.vector.tensor_copy(post[:, w * NCH + cc, :], tp[:, 0:P])

        # ---------- attention windows ----------
        for w in range(NWIN):
            qn = win.tile([P, NCH, P], F32, tag="qn")
            kn = win.tile([P, NCH, P], F32, tag="kn")
            for cc in range(NCH):
                nc.sync.dma_start(
                    qn[:, cc, :], q3[16 * cc : 16 * cc + 16, SS * w : SS * w + SS, :]
                )
                nc.sync.dma_start(
                    kn[:, cc, :], k3[16 * cc : 16 * cc + 16, SS * w : SS * w + SS, :]
                )
            # Zero-padded channel layout: each head's 16 channels padded to 32
            # (zeros in the upper half). After transposing, head h's rows sit
            # at a 32-aligned partition base with zeros alongside, so a plain
            # [32, :] slice is a correctly-masked per-head matmul operand.
            # Layout: [token, (group g in 2, head-in-group s in 4, 32)] bf16.
            qnp = win.tile([P, NCH, NH, 32], BF16, tag="qnp")
            knp = win.tile([P, NCH, NH, 32], BF16, tag="knp")
            nc.vector.memset(qnp[:, :, :, 16:32], 0.0)
            nc.vector.memset(knp[:, :, :, 16:32], 0.0)
            for cc in range(NCH):
                nc.vector.tensor_copy(
                    qnp[:, cc, :, 0:16],
                    qn[:, cc, :].rearrange("p (n d) -> p n d", d=HD),
                )
                nc.vector.tensor_copy(
                    knp[:, cc, :, 0:16],
                    kn[:, cc, :].rearrange("p (n d) -> p n d", d=HD),
                )
            # Transpose padded layouts: 2 groups of 4 heads per chunk.
            qtp = win.tile([P, 2, NCH, P], BF16, tag="qtp")
            ktp = win.tile([P, 2, NCH, P], BF16, tag="ktp")
            for cc in range(NCH):
                for g in range(2):
                    tp = scr_pool.tile([P, 480], BF16, tag="scr")
                    nc.tensor.transpose(
                        tp[:, 0:P],
                        qnp[:, cc, 4 * g : 4 * g + 4, :],
                        identb,
                    )
                    nc.vector.tensor_copy(qtp[:, g, cc, :], tp[:, 0:P])
                    tp2 = scr_pool.tile([P, 480], BF16, tag="scr")
                    nc.tensor.transpose(
                        tp2[:, 0:P],
                        knp[:, cc, 4 * g : 4 * g + 4, :],
                        identb,
                    )
                    nc.vector.tensor_copy(ktp[:, g, cc, :], tp2[:, 0:P])
            # V in window layout (bf16), padded to 32 cols per head:
            # cols 0:16 = V, col 16 = ones (softmax denominators), rest zero.
            va = win.tile([P, NCH, NH, 32], BF16, tag="va")
            nc.vector.memset(va[:, :, :, HD + 1 : 32], 0.0)
            for cc in range(NCH):
                nc.vector.tensor_copy(
                    va[:, cc, :, 0:HD],
                    vn[:, w * NCH + cc, :].rearrange("p (n d) -> p n d", d=HD),
                )
            nc.vector.memset(va[:, :, :, HD : HD + 1], 1.0)

            # S^T = K_w Q_w^T per head (contraction over hd on partitions),
            # then P^T = exp(scale * S^T) in bf16
            # S^T per head: 4 heads of a group run as concurrent row-tiles
            # (distinct 32-row groups, distinct PSUM banks), then one exp op
            # covers the whole group (FD=1536).
            pt = win.tile([P, NH * NCH, LW], BF16, tag="pt")
            pt_r = pt.rearrange("p (g2 s4 c3) t -> p g2 s4 c3 t", g2=2, s4=4)
            for g in range(2):
                for kc in range(NCH):
                    sp = sp_pool.tile([P, 4, 512], F32, tag="sp")
                    for sl in range(4):
                        nc.tensor.matmul(
                            sp[:, sl, 0:LW],
                            ktp[32 * sl : 32 * sl + 32, g, kc, :],
                            qtp[32 * sl : 32 * sl + 32, g, :, :],
                            start=True,
                            stop=True,
                            tile_position=(32 * sl, 0),
                        )
                    nc.scalar.activation(
                        pt_r[:, g, :, kc, :],
                        sp[:, :, 0:LW],
                        AF.Exp,
                        scale=SCALE,
                    )

            # U^T = V_aug.T @ P^T: stationary is the tiny V slice (17 cols),
            # moving operand is P^T with N=384. One matmul per (head, ki
            # chunk); 4 heads share a PSUM bank at 32-aligned column bases.
            uts = []
            for bank in range(2):
                ut = up_pool.tile([P, LW], F32, tag="ut")
                uts.append(ut)
                for j in range(4):
                    h = 4 * bank + j
                    for kc in range(NCH):
                        nc.tensor.matmul(
                            ut[32 * j : 32 * j + 32, :],
                            va[:, kc, h, :],
                            pt[:, NCH * h + kc, :],
                            start=(kc == 0),
                            stop=(kc == NCH - 1),
                            tile_position=(0, 32 * j),
                        )
            utb = []
            for bank in range(2):
                ub = win.tile([P, LW], BF16, tag="utb")
                nc.vector.tensor_copy(ub, uts[bank])
                utb.append(ub)
            # transpose U^T back to [qi, (head, d)], normalize, add pos, store
            for qc in range(NCH):
                tts = []
                for bank in range(2):
                    tt = scr_pool.tile([P, 480], BF16, tag="scr")
                    nc.tensor.transpose(
                        tt[:, 0:P], utb[bank][:, P * qc : P * (qc + 1)], identb
                    )
                    tts.append(tt)
                xn = xnp.tile([P, NH, HD], F32, tag="xn")
                for bank in range(2):
                    ttr = tts[bank][:, 0:P].rearrange("p (j x) -> p j x", x=32)
                    r4 = xnp.tile([P, 4], F32, tag="r4")
                    nc.vector.reciprocal(r4, ttr[:, :, HD : HD + 1])
                    nc.vector.tensor_mul(
                        xn[:, 4 * bank : 4 * bank + 4, :],
                        ttr[:, :, 0:HD],
                        r4[:, :, None].to_broadcast([P, 4, HD]),
                    )
                nc.vector.tensor_add(
                    xn, xn, post[:, w * NCH + qc, :].rearrange("p (n d) -> p n d", d=HD)
                )
                nc.sync.dma_start(
                    o3[16 * qc : 16 * qc + 16, SS * w : SS * w + SS, :],
                    xn.rearrange("p n d -> p (n d)"),
                )


def build_program():
    nc = bass.Bass("TRN2", target_bir_lowering=False)
    q = nc.dram_tensor("q", [BS, L, C], F32, kind="ExternalInput")
    k = nc.dram_tensor("k", [BS, L, C], F32, kind="ExternalInput")
    v = nc.dram_tensor("v", [BS, L, C], F32, kind="ExternalInput")
    p1wT = nc.dram_tensor("p1wT", [C, C // 4], F32, kind="ExternalInput")
    eb1 = nc.dram_tensor("eb1", [C // 4, 1], F32, kind="ExternalInput")
    p2wT = nc.dram_tensor("p2wT", [C // 4, G * C], F32, kind="ExternalInput")
    p2br = nc.dram_tensor("p2br", [C, G], F32, kind="ExternalInput")
    dwb = nc.dram_tensor("dwb", [C, G, 10], F32, kind="ExternalInput")
    out = nc.dram_tensor("out", [BS, L, C], F32, kind="ExternalOutput")
    with tile.TileContext(nc) as tc:
        with ExitStack() as ctx:
            _emit(ctx, tc, q[:], k[:], v[:], p1wT[:], eb1[:], p2wT[:], p2br[:],
                  dwb[:], out[:])
    _split_sync_waits(nc)
    nc.finalize()
    return nc


def host_weights(inputs):
    """Host-side weight preprocessing (BN folding, transposes, layout)."""
    p1w = np.asarray(inputs["proj1_w"], np.float32)      # (32, 128)
    p1b = np.asarray(inputs["proj1_b"], np.float32)      # (32,)
    gam = np.asarray(inputs["bn_gamma"], np.float32)
    bet = np.asarray(inputs["bn_beta"], np.float32)
    rm = np.asarray(inputs["bn_mean"], np.float32)
    rv = np.asarray(inputs["bn_var"], np.float32)
    p2w = np.asarray(inputs["proj2_w"], np.float32)      # (512, 32)
    p2b = np.asarray(inputs["proj2_b"], np.float32)      # (512,)
    dyn_w = np.asarray(inputs["dyn_w"], np.float32)      # (4, 128, 3, 3)
    dyn_b = np.asarray(inputs["dyn_b"], np.float32)      # (4, 128)

    es = gam / np.sqrt(rv + 1e-5)                        # (32,)
    p1w_eff = p1w * es[:, None]
    eb1 = ((p1b - rm) * es + bet).reshape(C // 4, 1)
    p1wT = np.ascontiguousarray(p1w_eff.T)               # (128, 32)
    p2wT = np.ascontiguousarray((0.5 * p2w).T)           # (32, 512); 0.5 from gelu
    p2br = np.ascontiguousarray(p2b.reshape(G, C).T)     # (128, 4)
    dwr = np.ascontiguousarray(dyn_w.transpose(1, 0, 2, 3).reshape(C, G, 9))
    dbr = np.ascontiguousarray(dyn_b.T)                  # (128, 4)
    dwb = np.ascontiguousarray(
        np.concatenate([dwr, dbr[:, :, None]], axis=2)   # (128, 4, 10)
    )
    return dict(p1wT=p1wT, eb1=eb1, p2wT=p2wT, p2br=p2br, dwb=dwb)


_PROGRAM = None


def get_program():
    global _PROGRAM
    if _PROGRAM is None:
        _PROGRAM = build_program()
    return _PROGRAM


def kernel(**inputs) -> np.ndarray:
    from concourse.bass_utils import run_bass_kernel_spmd

    q = np.ascontiguousarray(np.asarray(inputs["q"], np.float32))
    k = np.ascontiguousarray(np.asarray(inputs["k"], np.float32))
    v = np.ascontiguousarray(np.asarray(inputs["v"], np.float32))
    wts = host_weights(inputs)

    nc = get_program()
    in_maps = []
    for i in range(N_CORES):
        sl = slice(BS * i, BS * (i + 1))
        in_maps.append(
            {"q": q[sl], "k": k[sl], "v": v[sl], **wts}
        )
    res = run_bass_kernel_spmd(
        nc, in_maps, list(range(N_CORES)),
        trace=bool(int(os.environ.get("KERNEL_TRACE", "0"))),
    )
    out = np.concatenate([res.results[i]["out"] for i in range(N_CORES)], axis=0)
    return np.ascontiguousarray(out.astype(np.float32))


# revision 26
# speedup vs baseline: 1.0009x; 1.0009x over previous
"""Bass/Trainium2 kernel for nn_EDA_Attention (CSWin-style strip attention +
dynamic depthwise-conv positional branch).

Contract: kernel(**inputs) takes the FULL unsharded inputs (B=16) and returns
the FULL output (16, 2304, 128) float32. Internally shards data-parallel over
B across 8 NeuronCores (2 samples per core), runs one SPMD Bass program via
run_bass_kernel_spmd, and concatenates the per-core outputs.
"""

import os
from contextlib import ExitStack

import numpy as np

import concourse.bass as bass
import concourse.mybir as mybir
import concourse.tile as tile
from concourse.masks import make_identity

F32 = mybir.dt.float32

# The pinned walrus rejects instructions carrying more than one semaphore
# wait ("Too many sync wait commands"). After Tile scheduling, hoist excess
# waits onto same-engine NOPs inserted immediately before the instruction.
# Safe: each engine queue executes in order, and Tile only emits waits on
# semaphores whose increments were already enqueued.
def _split_sync_waits(nc, max_waits=1):
    counter = [0]
    for func in nc.m.functions:
        for blk in func.blocks:
            new_insts = []
            changed = False
            for inst in blk.instructions:
                si = inst.sync_info
                if si is not None and len(si.on_wait) > max_waits:
                    waits = list(si.on_wait)
                    keep, extra = waits[-max_waits:], waits[:-max_waits]
                    for w in extra:
                        counter[0] += 1
                        nop = mybir.InstNoOp(
                            name=f"I-wsplit-{counter[0]}", ins=[], outs=[]
                        )
                        nop.engine = inst.engine
                        nop.sync_info = mybir.SyncInfo(on_wait=[w], on_update=[])
                        nc.register_instruction(nop, overwrite=True)
                        new_insts.append(nop)
                    inst.sync_info = mybir.SyncInfo(
                        on_wait=keep, on_update=list(si.on_update)
                    )
                    changed = True
                new_insts.append(inst)
            if changed:
                blk.instructions = new_insts
F32R = mybir.dt.float32r
BF16 = mybir.dt.bfloat16
AF = mybir.ActivationFunctionType
ALU = mybir.AluOpType

# Problem constants (hardcoded per spec).
B_FULL = 16
N_CORES = 8
BS = B_FULL // N_CORES  # samples per core
H = 48
W = 48
C = 128
L = H * W               # 2304
G = 4
NH = 8
HD = C // NH            # 16
SS = 8                  # strip (window) width; idx=0 -> vertical strips
NWIN = W // SS          # 6 windows per sample
LW = H * SS             # 384 tokens per window
NCH = LW // 128         # 3 chunks of 128 tokens
KK = 3                  # conv kernel size
SCALE = HD ** -0.5      # attention scale (0.25)
GELU_A = 0.7978845608028654  # sqrt(2/pi)
GELU_B = 0.044715

P = 128


def _emit(ctx: ExitStack, tc, q, k, v, p1wT, eb1, p2wT, p2br, dwb, out):
    nc = tc.nc

    singles = ctx.enter_context(tc.tile_pool(name="singles", bufs=1))
    samp = ctx.enter_context(tc.tile_pool(name="samp", bufs=2))
    win = ctx.enter_context(tc.tile_pool(name="win", bufs=3))
    xnp = ctx.enter_context(tc.tile_pool(name="xnp", bufs=6))
    smalls = ctx.enter_context(tc.tile_pool(name="smalls", bufs=2))
    sp_pool = ctx.enter_context(tc.tile_pool(name="sp", bufs=1, space="PSUM"))
    up_pool = ctx.enter_context(tc.tile_pool(name="up", bufs=2, space="PSUM"))
    scr_pool = ctx.enter_context(tc.tile_pool(name="scr", bufs=2, space="PSUM"))

    ident = singles.tile([P, P], F32)
    make_identity(nc, ident)
    identb = singles.tile([P, P], BF16, tag="identb")
    nc.vector.tensor_copy(identb, ident)
    p1wT_sb = singles.tile([P, C // 4], F32, tag="p1wT")
    nc.sync.dma_start(p1wT_sb, p1wT)
    eb1_sb = singles.tile([C // 4, 1], F32, tag="eb1")
    nc.sync.dma_start(eb1_sb, eb1)
    p2wT_sb = singles.tile([C // 4, G * C], F32, tag="p2wT")
    nc.sync.dma_start(p2wT_sb, p2wT)
    p2br_sb = singles.tile([P, G], F32, tag="p2br")
    nc.sync.dma_start(p2br_sb, p2br)
    dwb_sb = singles.tile([P, G, 10], F32, tag="dwb")
    nc.sync.dma_start(dwb_sb, dwb)

    for s in range(BS):
        q3 = q[s].rearrange("(h w) c -> h w c", w=W)
        k3 = k[s].rearrange("(h w) c -> h w c", w=W)
        v3 = v[s].rearrange("(h w) c -> h w c", w=W)
        o3 = out[s].rearrange("(h w) c -> h w c", w=W)

        # ---------- sample prologue: v in natural + image layout ----------
        vn = samp.tile([P, NWIN * NCH, P], F32, tag="vn")
        for w in range(NWIN):
            for cc in range(NCH):
                nc.sync.dma_start(
                    vn[:, w * NCH + cc, :],
                    v3[16 * cc : 16 * cc + 16, SS * w : SS * w + SS, :],
                )

        # zero-padded image [C, H+2, W+2] for the 3x3 conv (bf16)
        vnb = samp.tile([P, NWIN * NCH, P], BF16, tag="vnb")
        nc.vector.tensor_copy(vnb, vn)
        vpad = samp.tile([P, H + 2, W + 2], BF16, tag="vpad")
        nc.vector.memset(vpad[:, 0, :], 0.0)
        nc.vector.memset(vpad[:, H + 1, :], 0.0)
        nc.vector.memset(vpad[:, 1 : H + 1, 0:1], 0.0)
        nc.vector.memset(vpad[:, 1 : H + 1, W + 1 : W + 2], 0.0)
        for w in range(NWIN):
            for cc in range(NCH):
                tp = scr_pool.tile([P, 480], BF16, tag="scr")
                nc.tensor.transpose(tp[:, 0:P], vnb[:, w * NCH + cc, :], identb)
                nc.vector.tensor_copy(
                    vpad[:, 1 + 16 * cc : 1 + 16 * cc + 16, 1 + SS * w : 1 + SS * w + SS],
                    tp[:, 0:P].rearrange("p (a b) -> p a b", b=SS),
                )

        # ---------- dynamic conv weight branch ----------
        # pooled 3x3 block sums (AdaptiveAvgPool to 3x3 -> / 256)
        pooled = smalls.tile([P, KK, KK], F32, tag="pooled")
        nc.vector.tensor_reduce(
            out=pooled,
            in_=vpad[:, 1 : H + 1, 1 : W + 1].rearrange(
                "p (a y) (b x) -> p a b y x", y=16, x=16
            ),
            axis=mybir.AxisListType.XY,
            op=ALU.add,
        )
        rhs10 = smalls.tile([P, 10], F32, tag="rhs10")
        nc.vector.tensor_scalar(
            out=rhs10[:, 0:9],
            in0=pooled.rearrange("p a b -> p (a b)"),
            scalar1=1.0 / 256.0,
            scalar2=None,
            op0=ALU.mult,
        )
        gsum = smalls.tile([P, 1], F32, tag="gsum")
        nc.vector.tensor_reduce(
            out=gsum, in_=pooled, axis=mybir.AxisListType.XY, op=ALU.add
        )
        nc.vector.tensor_scalar(
            out=rhs10[:, 9:10], in0=gsum, scalar1=1.0 / float(L), scalar2=None,
            op0=ALU.mult,
        )
        # proj1 (BN folded host-side) -> gelu(tanh approx; x0.5 folded into p2w)
        pj1 = scr_pool.tile([P, 480], F32, tag="scr")
        nc.tensor.matmul(pj1[0 : C // 4, 0:10], p1wT_sb, rhs10, start=True, stop=True)
        g0 = smalls.tile([C // 4, 10], F32, tag="g0")
        nc.vector.tensor_scalar(
            out=g0, in0=pj1[0 : C // 4, 0:10], scalar1=eb1_sb, scalar2=None,
            op0=ALU.add,
        )
        x2 = smalls.tile([C // 4, 10], F32, tag="x2")
        nc.vector.tensor_mul(x2, g0, g0)
        x3 = smalls.tile([C // 4, 10], F32, tag="x3")
        nc.vector.tensor_mul(x3, x2, g0)
        inner = smalls.tile([C // 4, 10], F32, tag="inner")
        nc.vector.scalar_tensor_tensor(
            out=inner, in0=x3, scalar=GELU_B, in1=g0, op0=ALU.mult, op1=ALU.add
        )
        th = smalls.tile([C // 4, 10], F32, tag="th")
        nc.scalar.activation(th, inner, AF.Tanh, scale=GELU_A)
        ge = smalls.tile([C // 4, 10], F32, tag="ge")
        nc.vector.scalar_tensor_tensor(
            out=ge, in0=th, scalar=1.0, in1=g0, op0=ALU.add, op1=ALU.mult
        )
        # proj2 per group, + bias
        s_sb = smalls.tile([P, G, 10], F32, tag="s_sb")
        for g in range(G):
            pj2 = scr_pool.tile([P, 480], F32, tag="scr")
            nc.tensor.matmul(
                pj2[:, 0:10], p2wT_sb[:, P * g : P * (g + 1)], ge,
                start=True, stop=True,
            )
            nc.vector.tensor_scalar(
                out=s_sb[:, g, :], in0=pj2[:, 0:10], scalar1=p2br_sb[:, g : g + 1],
                scalar2=None, op0=ALU.add,
            )
        # softmax over G (no max subtraction; values are small) + weighted sum
        e_sb = smalls.tile([P, G, 10], F32, tag="e_sb")
        nc.scalar.activation(e_sb, s_sb, AF.Exp)
        esum = smalls.tile([P, 10], F32, tag="esum")
        nc.vector.tensor_add(esum, e_sb[:, 0, :], e_sb[:, 1, :])
        nc.vector.tensor_add(esum, esum, e_sb[:, 2, :])
        nc.vector.tensor_add(esum, esum, e_sb[:, 3, :])
        rs = smalls.tile([P, 10], F32, tag="rs")
        nc.vector.reciprocal(rs, esum)
        wnum = smalls.tile([P, 10], F32, tag="wnum")
        tmpg = smalls.tile([P, 10], F32, tag="tmpg")
        nc.vector.tensor_mul(wnum, e_sb[:, 0, :], dwb_sb[:, 0, :])
        for g in range(1, G):
            nc.vector.tensor_mul(tmpg, e_sb[:, g, :], dwb_sb[:, g, :])
            nc.vector.tensor_add(wnum, wnum, tmpg)
        wb = smalls.tile([P, 10], F32, tag="wb")  # cols 0:9 taps, col 9 bias
        nc.vector.tensor_mul(wb, wnum, rs)
        # diagonal weight tiles for the depthwise conv-as-matmul
        diag9 = samp.tile([P, 9, P], BF16, tag="diag9")
        for t in range(9):
            nc.vector.tensor_scalar(
                out=diag9[:, t, :], in0=ident, scalar1=wb[:, t : t + 1],
                scalar2=None, op0=ALU.mult,
            )
        # depthwise 3x3 conv: 9 shifted diag-matmuls accumulated in PSUM.
        # Output written straight into window-chunk-major layout so the
        # pos transposes below read contiguous [128, 128] slices.
        pos_wcm = samp.tile([P, NWIN * NCH, P], BF16, tag="pos_wcm")
        povw = pos_wcm.rearrange(
            "p (w c2) (h2 r s) -> p w c2 h2 r s", c2=NCH, r=8, s=SS
        )
        for cc in range(NCH):
            for hf in range(2):
                r0 = 16 * cc + 8 * hf
                cp = scr_pool.tile([P, 512], F32, tag="scr")
                for t in range(9):
                    dy, dx = t // 3 - 1, t % 3 - 1
                    nc.tensor.matmul(
                        cp[:, 0 : 8 * W],
                        diag9[:, t, :],
                        vpad[:, 1 + r0 + dy : 1 + r0 + dy + 8, 1 + dx : 1 + dx + W],
                        start=(t == 0),
                        stop=(t == 8),
                    )
                nc.vector.tensor_scalar(
                    out=povw[:, :, cc, hf, :, :].rearrange("p w r s -> p r w s"),
                    in0=cp[:, 0 : 8 * W].rearrange("p (r w s) -> p r w s", w=NWIN, s=SS),
                    scalar1=wb[:, 9:10],
                    scalar2=None,
                    op0=ALU.add,
                )
        # transpose pos back to [token, C] per window chunk
        post = samp.tile([P, NWIN * NCH, P], F32, tag="post")
        for w in range(NWIN):
            for cc in range(NCH):
                tp = scr_pool.tile([P, 480], BF16, tag="scr")
                nc.tensor.transpose(
                    tp[:, 0:P],
                    pos_wcm[:, w * NCH + cc, :],
                    identb,
                )
                nc.vector.tensor_copy(post[:, w * NCH + cc, :], tp[:, 0:P])

        # ---------- attention windows ----------
        for w in range(NWIN):
            qn = win.tile([P, NCH, P], F32, tag="qn")
            kn = win.tile([P, NCH, P], F32, tag="kn")
            for cc in range(NCH):
                nc.sync.dma_start(
                    qn[:, cc, :], q3[16 * cc : 16 * cc + 16, SS * w : SS * w + SS, :]
                )
                nc.sync.dma_start(
                    kn[:, cc, :], k3[16 * cc : 16 * cc + 16, SS * w : SS * w + SS, :]
                )
            # Zero-padded channel layout: each head's 16 channels padded to 32
            # (zeros in the upper half). After transposing, head h's rows sit
            # at a 32-aligned partition base with zeros alongside, so a plain
            # [32, :] slice is a correctly-masked per-head matmul operand.
            # Layout: [token, (group g in 2, head-in-group s in 4, 32)] bf16.
            qnp = win.tile([P, NCH, NH, 32], BF16, tag="qnp")
            knp = win.tile([P, NCH, NH, 32], BF16, tag="knp")
            nc.vector.memset(qnp[:, :, :, 16:32], 0.0)
            nc.vector.memset(knp[:, :, :, 16:32], 0.0)
            for cc in range(NCH):
                nc.vector.tensor_copy(
                    qnp[:, cc, :, 0:16],
                    qn[:, cc, :].rearrange("p (n d) -> p n d", d=HD),
                )
                nc.vector.tensor_copy(
                    knp[:, cc, :, 0:16],
                    kn[:, cc, :].rearrange("p (n d) -> p n d", d=HD),
                )
            # Transpose padded layouts: 2 groups of 4 heads per chunk.
            qtp = win.tile([P, 2, NCH, P], BF16, tag="qtp")
            ktp = win.tile([P, 2, NCH, P], BF16, tag="ktp")
            for cc in range(NCH):
                for g in range(2):
                    tp = scr_pool.tile([P, 480], BF16, tag="scr")
                    nc.tensor.transpose(
                        tp[:, 0:P],
                        qnp[:, cc, 4 * g : 4 * g + 4, :],
                        identb,
                    )
                    nc.vector.tensor_copy(qtp[:, g, cc, :], tp[:, 0:P])
                    tp2 = scr_pool.tile([P, 480], BF16, tag="scr")
                    nc.tensor.transpose(
                        tp2[:, 0:P],
                        knp[:, cc, 4 * g : 4 * g + 4, :],
                        identb,
                    )
                    nc.vector.tensor_copy(ktp[:, g, cc, :], tp2[:, 0:P])
            # V in window layout (bf16), padded to 32 cols per head:
            # cols 0:16 = V, col 16 = ones (softmax denominators), rest zero.
            va = win.tile([P, NCH, NH, 32], BF16, tag="va")
            nc.vector.memset(va[:, :, :, HD + 1 : 32], 0.0)
            for cc in range(NCH):
                nc.vector.tensor_copy(
                    va[:, cc, :, 0:HD],
                    vn[:, w * NCH + cc, :].rearrange("p (n d) -> p n d", d=HD),
                )
            nc.vector.memset(va[:, :, :, HD : HD + 1], 1.0)

            # S^T = K_w Q_w^T per head (contraction over hd on partitions),
            # then P^T = exp(scale * S^T) in bf16
            # S^T per head: 4 heads of a group run as concurrent row-tiles
            # (distinct 32-row groups, distinct PSUM banks), then one exp op
            # covers the whole group (FD=1536).
            pt = win.tile([P, NH * NCH, LW], BF16, tag="pt")
            pt_r = pt.rearrange("p (g2 s4 c3) t -> p g2 s4 c3 t", g2=2, s4=4)
            for g in range(2):
                for kc in range(NCH):
                    sp = sp_pool.tile([P, 4, 512], F32, tag="sp")
                    for sl in range(4):
                        nc.tensor.matmul(
                            sp[:, sl, 0:LW],
                            ktp[32 * sl : 32 * sl + 32, g, kc, :],
                            qtp[32 * sl : 32 * sl + 32, g, :, :],
                            start=True,
                            stop=True,
                            tile_position=(32 * sl, 0),
                        )
                    nc.scalar.activation(
                        pt_r[:, g, :, kc, :],
                        sp[:, :, 0:LW],
                        AF.Exp,
                        scale=SCALE,
                    )

            # U^T = V_aug.T @ P^T: stationary is the tiny V slice (17 cols),
            # moving operand is P^T with N=384. One matmul per (head, ki
            # chunk); 4 heads share a PSUM bank at 32-aligned column bases.
            uts = []
            for bank in range(2):
                ut = up_pool.tile([P, LW], F32, tag="ut")
                uts.append(ut)
                for j in range(4):
                    h = 4 * bank + j
                    for kc in range(NCH):
                        nc.tensor.matmul(
                            ut[32 * j : 32 * j + 32, :],
                            va[:, kc, h, :],
                            pt[:, NCH * h + kc, :],
                            start=(kc == 0),
                            stop=(kc == NCH - 1),
                            tile_position=(0, 32 * j),
                        )
            utb = []
            for bank in range(2):
                ub = win.tile([P, LW], BF16, tag="utb")
                nc.vector.tensor_copy(ub, uts[bank])
                utb.append(ub)
            # transpose U^T back to [qi, (head, d)], normalize, add pos, store
            for qc in range(NCH):
                tts = []
                for bank in range(2):
                    tt = scr_pool.tile([P, 480], BF16, tag="scr")
                    nc.tensor.transpose(
                        tt[:, 0:P], utb[bank][:, P * qc : P * (qc + 1)], identb
                    )
                    tts.append(tt)
                xn = xnp.tile([P, NH, HD], F32, tag="xn")
                for bank in range(2):
                    ttr = tts[bank][:, 0:P].rearrange("p (j x) -> p j x", x=32)
                    r4 = xnp.tile([P, 4], F32, tag="r4")
                    nc.vector.reciprocal(r4, ttr[:, :, HD : HD + 1])
                    nc.vector.tensor_mul(
                        xn[:, 4 * bank : 4 * bank + 4, :],
                        ttr[:, :, 0:HD],
                        r4[:, :, None].to_broadcast([P, 4, HD]),
                    )
                nc.vector.tensor_add(
                    xn, xn, post[:, w * NCH + qc, :].rearrange("p (n d) -> p n d", d=HD)
                )
                nc.sync.dma_start(
                    o3[16 * qc : 16 * qc + 16, SS * w : SS * w + SS, :],
                    xn.rearrange("p n d -> p (n d)"),
                )


def build_program():
    nc = bass.Bass("TRN2", target_bir_lowering=False)
    q = nc.dram_tensor("q", [BS, L, C], F32, kind="ExternalInput")
    k = nc.dram_tensor("k", [BS, L, C], F32, kind="ExternalInput")
    v = nc.dram_tensor("v", [BS, L, C], F32, kind="ExternalInput")
    p1wT = nc.dram_tensor("p1wT", [C, C // 4], F32, kind="ExternalInput")
    eb1 = nc.dram_tensor("eb1", [C // 4, 1], F32, kind="ExternalInput")
    p2wT = nc.dram_tensor("p2wT", [C // 4, G * C], F32, kind="ExternalInput")
    p2br = nc.dram_tensor("p2br", [C, G], F32, kind="ExternalInput")
    dwb = nc.dram_tensor("dwb", [C, G, 10], F32, kind="ExternalInput")
    out = nc.dram_tensor("out", [BS, L, C], F32, kind="ExternalOutput")
    with tile.TileContext(nc) as tc:
        with ExitStack() as ctx:
            _emit(ctx, tc, q[:], k[:], v[:], p1wT[:], eb1[:], p2wT[:], p2br[:],
                  dwb[:], out[:])
    _split_sync_waits(nc)
    nc.finalize()
    return nc


def host_weights(inputs):
    """Host-side weight preprocessing (BN folding, transposes, layout)."""
    p1w = np.asarray(inputs["proj1_w"], np.float32)      # (32, 128)
    p1b = np.asarray(inputs["proj1_b"], np.float32)      # (32,)
    gam = np.asarray(inputs["bn_gamma"], np.float32)
    bet = np.asarray(inputs["bn_beta"], np.float32)
    rm = np.asarray(inputs["bn_mean"], np.float32)
    rv = np.asarray(inputs["bn_var"], np.float32)
    p2w = np.asarray(inputs["proj2_w"], np.float32)      # (512, 32)
    p2b = np.asarray(inputs["proj2_b"], np.float32)      # (512,)
    dyn_w = np.asarray(inputs["dyn_w"], np.float32)      # (4, 128, 3, 3)
    dyn_b = np.asarray(inputs["dyn_b"], np.float32)      # (4, 128)

    es = gam / np.sqrt(rv + 1e-5)                        # (32,)
    p1w_eff = p1w * es[:, None]
    eb1 = ((p1b - rm) * es + bet).reshape(C // 4, 1)
    p1wT = np.ascontiguousarray(p1w_eff.T)               # (128, 32)
    p2wT = np.ascontiguousarray((0.5 * p2w).T)           # (32, 512); 0.5 from gelu
    p2br = np.ascontiguousarray(p2b.reshape(G, C).T)     # (128, 4)
    dwr = np.ascontiguousarray(dyn_w.transpose(1, 0, 2, 3).reshape(C, G, 9))
    dbr = np.ascontiguousarray(dyn_b.T)                  # (128, 4)
    dwb = np.ascontiguousarray(
        np.concatenate([dwr, dbr[:, :, None]], axis=2)   # (128, 4, 10)
    )
    return dict(p1wT=p1wT, eb1=eb1, p2wT=p2wT, p2br=p2br, dwb=dwb)


_PROGRAM = None


def get_program():
    global _PROGRAM
    if _PROGRAM is None:
        _PROGRAM = build_program()
    return _PROGRAM


def kernel(**inputs) -> np.ndarray:
    from concourse.bass_utils import run_bass_kernel_spmd

    q = np.ascontiguousarray(np.asarray(inputs["q"], np.float32))
    k = np.ascontiguousarray(np.asarray(inputs["k"], np.float32))
    v = np.ascontiguousarray(np.asarray(inputs["v"], np.float32))
    wts = host_weights(inputs)

    nc = get_program()
    in_maps = []
    for i in range(N_CORES):
        sl = slice(BS * i, BS * (i + 1))
        in_maps.append(
            {"q": q[sl], "k": k[sl], "v": v[sl], **wts}
        )
    res = run_bass_kernel_spmd(
        nc, in_maps, list(range(N_CORES)),
        trace=bool(int(os.environ.get("KERNEL_TRACE", "0"))),
    )
    out = np.concatenate([res.results[i]["out"] for i in range(N_CORES)], axis=0)
    return np.ascontiguousarray(out.astype(np.float32))


# revision 27
# speedup vs baseline: 1.3729x; 1.3716x over previous
"""Bass/Trainium2 kernel for nn_EDA_Attention (CSWin-style strip attention +
dynamic depthwise-conv positional branch).

Contract: kernel(**inputs) takes the FULL unsharded inputs (B=16) and returns
the FULL output (16, 2304, 128) float32. Internally shards data-parallel over
B across 8 NeuronCores (2 samples per core), runs one SPMD Bass program via
run_bass_kernel_spmd, and concatenates the per-core outputs.

Host-side prep is pure layout/dtype (window-major reorder, per-head zero
padding, bf16 cast, folded BN constants); all arithmetic runs on device.
"""

import os
from contextlib import ExitStack

import ml_dtypes
import numpy as np

import concourse.bass as bass
import concourse.mybir as mybir
import concourse.tile as tile
from concourse.masks import make_identity

F32 = mybir.dt.float32
BF16 = mybir.dt.bfloat16
AF = mybir.ActivationFunctionType
ALU = mybir.AluOpType

# Problem constants (hardcoded per spec).
B_FULL = 16
N_CORES = 8
BS = B_FULL // N_CORES  # samples per core
H = 48
W = 48
C = 128
L = H * W               # 2304
G = 4
NH = 8
HD = C // NH            # 16
SS = 8                  # strip (window) width; idx=0 -> vertical strips
NWIN = W // SS          # 6 windows per sample
LW = H * SS             # 384 tokens per window
NCH = LW // 128         # 3 chunks of 128 tokens
KK = 3                  # conv kernel size
SCALE = HD ** -0.5      # attention scale (0.25)
GELU_A = 0.7978845608028654  # sqrt(2/pi)
GELU_B = 0.044715

P = 128


# The pinned walrus rejects instructions carrying more than one semaphore
# wait ("Too many sync wait commands"). After Tile scheduling, hoist excess
# waits onto same-engine NOPs inserted immediately before the instruction.
# Safe: each engine queue executes in order, and Tile only emits waits on
# semaphores whose increments were already enqueued.
def _split_sync_waits(nc, max_waits=1):
    counter = [0]
    for func in nc.m.functions:
        for blk in func.blocks:
            new_insts = []
            changed = False
            for inst in blk.instructions:
                si = inst.sync_info
                if si is not None and len(si.on_wait) > max_waits:
                    waits = list(si.on_wait)
                    keep, extra = waits[-max_waits:], waits[:-max_waits]
                    for w in extra:
                        counter[0] += 1
                        nop = mybir.InstNoOp(
                            name=f"I-wsplit-{counter[0]}", ins=[], outs=[]
                        )
                        nop.engine = inst.engine
                        nop.sync_info = mybir.SyncInfo(on_wait=[w], on_update=[])
                        nc.register_instruction(nop, overwrite=True)
                        new_insts.append(nop)
                    inst.sync_info = mybir.SyncInfo(
                        on_wait=keep, on_update=list(si.on_update)
                    )
                    changed = True
                new_insts.append(inst)
            if changed:
                blk.instructions = new_insts


def _emit(ctx: ExitStack, tc, qw, kw, vaw, vimg, p1wT, eb1, p2wT, p2br, dwb, out):
    nc = tc.nc

    singles = ctx.enter_context(tc.tile_pool(name="singles", bufs=1))
    samp = ctx.enter_context(tc.tile_pool(name="samp", bufs=2))
    win = ctx.enter_context(tc.tile_pool(name="win", bufs=3))
    xnp = ctx.enter_context(tc.tile_pool(name="xnp", bufs=6))
    smalls = ctx.enter_context(tc.tile_pool(name="smalls", bufs=2))
    sp_pool = ctx.enter_context(tc.tile_pool(name="sp", bufs=1, space="PSUM"))
    up_pool = ctx.enter_context(tc.tile_pool(name="up", bufs=2, space="PSUM"))
    scr_pool = ctx.enter_context(tc.tile_pool(name="scr", bufs=2, space="PSUM"))

    ident = singles.tile([P, P], F32)
    make_identity(nc, ident)
    identb = singles.tile([P, P], BF16, tag="identb")
    nc.vector.tensor_copy(identb, ident)
    p1wT_sb = singles.tile([P, C // 4], F32, tag="p1wT")
    nc.sync.dma_start(p1wT_sb, p1wT)
    eb1_sb = singles.tile([C // 4, 1], F32, tag="eb1")
    nc.sync.dma_start(eb1_sb, eb1)
    p2wT_sb = singles.tile([C // 4, G * C], F32, tag="p2wT")
    nc.sync.dma_start(p2wT_sb, p2wT)
    p2br_sb = singles.tile([P, G], F32, tag="p2br")
    nc.sync.dma_start(p2br_sb, p2br)
    dwb_sb = singles.tile([P, G, 10], F32, tag="dwb")
    nc.sync.dma_start(dwb_sb, dwb)

    for s in range(BS):
        o3 = out[s].rearrange("(h w) c -> h w c", w=W)

        # ---------- conv branch: padded image + dynamic weights ----------
        vpad = samp.tile([P, H + 2, W + 2], BF16, tag="vpad")
        nc.sync.dma_start(vpad, vimg[s])

        # pooled 3x3 block sums (AdaptiveAvgPool to 3x3 -> / 256)
        pooled = smalls.tile([P, KK, KK], F32, tag="pooled")
        nc.vector.tensor_reduce(
            out=pooled,
            in_=vpad[:, 1 : H + 1, 1 : W + 1].rearrange(
                "p (a y) (b x) -> p a b y x", y=16, x=16
            ),
            axis=mybir.AxisListType.XY,
            op=ALU.add,
        )
        rhs10 = smalls.tile([P, 10], F32, tag="rhs10")
        nc.vector.tensor_scalar(
            out=rhs10[:, 0:9],
            in0=pooled.rearrange("p a b -> p (a b)"),
            scalar1=1.0 / 256.0,
            scalar2=None,
            op0=ALU.mult,
        )
        gsum = smalls.tile([P, 1], F32, tag="gsum")
        nc.vector.tensor_reduce(
            out=gsum, in_=pooled, axis=mybir.AxisListType.XY, op=ALU.add
        )
        nc.vector.tensor_scalar(
            out=rhs10[:, 9:10], in0=gsum, scalar1=1.0 / float(L), scalar2=None,
            op0=ALU.mult,
        )
        # proj1 (BN folded host-side) -> gelu(tanh approx; x0.5 folded into p2w)
        pj1 = scr_pool.tile([P, 512], F32, tag="scr")
        nc.tensor.matmul(pj1[0 : C // 4, 0:10], p1wT_sb, rhs10, start=True, stop=True)
        g0 = smalls.tile([C // 4, 10], F32, tag="g0")
        nc.vector.tensor_scalar(
            out=g0, in0=pj1[0 : C // 4, 0:10], scalar1=eb1_sb, scalar2=None,
            op0=ALU.add,
        )
        x2 = smalls.tile([C // 4, 10], F32, tag="x2")
        nc.vector.tensor_mul(x2, g0, g0)
        x3 = smalls.tile([C // 4, 10], F32, tag="x3")
        nc.vector.tensor_mul(x3, x2, g0)
        inner = smalls.tile([C // 4, 10], F32, tag="inner")
        nc.vector.scalar_tensor_tensor(
            out=inner, in0=x3, scalar=GELU_B, in1=g0, op0=ALU.mult, op1=ALU.add
        )
        th = smalls.tile([C // 4, 10], F32, tag="th")
        nc.scalar.activation(th, inner, AF.Tanh, scale=GELU_A)
        ge = smalls.tile([C // 4, 10], F32, tag="ge")
        nc.vector.scalar_tensor_tensor(
            out=ge, in0=th, scalar=1.0, in1=g0, op0=ALU.add, op1=ALU.mult
        )
        # proj2 per group, + bias
        s_sb = smalls.tile([P, G, 10], F32, tag="s_sb")
        for g in range(G):
            pj2 = scr_pool.tile([P, 512], F32, tag="scr")
            nc.tensor.matmul(
                pj2[:, 0:10], p2wT_sb[:, P * g : P * (g + 1)], ge,
                start=True, stop=True,
            )
            nc.vector.tensor_scalar(
                out=s_sb[:, g, :], in0=pj2[:, 0:10], scalar1=p2br_sb[:, g : g + 1],
                scalar2=None, op0=ALU.add,
            )
        # softmax over G (no max subtraction; values are small) + weighted sum
        e_sb = smalls.tile([P, G, 10], F32, tag="e_sb")
        nc.scalar.activation(e_sb, s_sb, AF.Exp)
        esum = smalls.tile([P, 10], F32, tag="esum")
        nc.vector.tensor_add(esum, e_sb[:, 0, :], e_sb[:, 1, :])
        nc.vector.tensor_add(esum, esum, e_sb[:, 2, :])
        nc.vector.tensor_add(esum, esum, e_sb[:, 3, :])
        rs = smalls.tile([P, 10], F32, tag="rs")
        nc.vector.reciprocal(rs, esum)
        wnum = smalls.tile([P, 10], F32, tag="wnum")
        tmpg = smalls.tile([P, 10], F32, tag="tmpg")
        nc.vector.tensor_mul(wnum, e_sb[:, 0, :], dwb_sb[:, 0, :])
        for g in range(1, G):
            nc.vector.tensor_mul(tmpg, e_sb[:, g, :], dwb_sb[:, g, :])
            nc.vector.tensor_add(wnum, wnum, tmpg)
        wb = smalls.tile([P, 10], F32, tag="wb")  # cols 0:9 taps, col 9 bias
        nc.vector.tensor_mul(wb, wnum, rs)
        # diagonal weight tiles for the depthwise conv-as-matmul
        diag9 = samp.tile([P, 9, P], BF16, tag="diag9")
        for t in range(9):
            nc.vector.tensor_scalar(
                out=diag9[:, t, :], in0=ident, scalar1=wb[:, t : t + 1],
                scalar2=None, op0=ALU.mult,
            )
        # depthwise 3x3 conv: 9 shifted diag-matmuls accumulated in PSUM.
        # Output written straight into window-chunk-major layout so the
        # pos transposes below read contiguous [128, 128] slices.
        pos_wcm = samp.tile([P, NWIN * NCH, P], BF16, tag="pos_wcm")
        povw = pos_wcm.rearrange(
            "p (w c2) (h2 r s) -> p w c2 h2 r s", c2=NCH, r=8, s=SS
        )
        for cc in range(NCH):
            for hf in range(2):
                r0 = 16 * cc + 8 * hf
                cp = scr_pool.tile([P, 512], F32, tag="scr")
                for t in range(9):
                    dy, dx = t // 3 - 1, t % 3 - 1
                    nc.tensor.matmul(
                        cp[:, 0 : 8 * W],
                        diag9[:, t, :],
                        vpad[:, 1 + r0 + dy : 1 + r0 + dy + 8, 1 + dx : 1 + dx + W],
                        start=(t == 0),
                        stop=(t == 8),
                    )
                nc.vector.tensor_scalar(
                    out=povw[:, :, cc, hf, :, :].rearrange("p w r s -> p r w s"),
                    in0=cp[:, 0 : 8 * W].rearrange("p (r w s) -> p r w s", w=NWIN, s=SS),
                    scalar1=wb[:, 9:10],
                    scalar2=None,
                    op0=ALU.add,
                )
        # transpose pos back to [token, C] per window chunk
        post = samp.tile([P, NWIN * NCH, P], F32, tag="post")
        for w in range(NWIN):
            for cc in range(NCH):
                tp = scr_pool.tile([P, 512], BF16, tag="scr")
                nc.tensor.transpose(
                    tp[:, 0:P],
                    pos_wcm[:, w * NCH + cc, :],
                    identb,
                )
                nc.vector.tensor_copy(post[:, w * NCH + cc, :], tp[:, 0:P])

        # ---------- attention windows ----------
        for w in range(NWIN):
            # Q^T / K^T loaded directly via DMA XBAR transpose from the
            # host-prepped window-major, per-head zero-padded bf16 arrays.
            # Head h = 4g + j sits at partitions 32j..32j+16 of group g,
            # zeros at 32j+16..32j+32 — so a [32, :] slice is a correctly
            # masked per-head matmul operand.
            qtp = win.tile([P, 2, LW], BF16, tag="qtp")
            ktp = win.tile([P, 2, LW], BF16, tag="ktp")
            for g in range(2):
                nc.sync.dma_start(qtp[:, g, :], qw[s, w, :, g, :], transpose=True)
                nc.sync.dma_start(ktp[:, g, :], kw[s, w, :, g, :], transpose=True)
            # V in window layout: [ki, head, 32] with col 16 = ones (softmax
            # denominators), rest zero — host-prepped.
            va = win.tile([P, NCH, NH, 32], BF16, tag="va")
            for kc in range(NCH):
                nc.sync.dma_start(va[:, kc, :, :], vaw[s, w, P * kc : P * (kc + 1), :, :])

            # S^T per head: 4 heads of a group run as concurrent row-tiles
            # (distinct 32-row groups, distinct PSUM banks), then one exp op
            # covers the whole group (FD=1536).
            pt = win.tile([P, NH * NCH, LW], BF16, tag="pt")
            pt_r = pt.rearrange("p (g2 s4 c3) t -> p g2 s4 c3 t", g2=2, s4=4)
            for g in range(2):
                for kc in range(NCH):
                    sp = sp_pool.tile([P, 4, 512], F32, tag="sp")
                    for sl in range(4):
                        nc.tensor.matmul(
                            sp[:, sl, 0:LW],
                            ktp[32 * sl : 32 * sl + 32, g, P * kc : P * (kc + 1)],
                            qtp[32 * sl : 32 * sl + 32, g, :],
                            start=True,
                            stop=True,
                            tile_position=(32 * sl, 0),
                        )
                    nc.scalar.activation(
                        pt_r[:, g, :, kc, :],
                        sp[:, :, 0:LW],
                        AF.Exp,
                        scale=SCALE,
                    )

            # U^T = V_aug.T @ P^T: stationary is the tiny V slice (32 cols),
            # moving operand is P^T with N=384. One matmul per (head, ki
            # chunk); 4 heads share a PSUM bank at 32-aligned column bases.
            uts = []
            for bank in range(2):
                ut = up_pool.tile([P, LW], F32, tag="ut")
                uts.append(ut)
                for j in range(4):
                    h = 4 * bank + j
                    for kc in range(NCH):
                        nc.tensor.matmul(
                            ut[32 * j : 32 * j + 32, :],
                            va[:, kc, h, :],
                            pt[:, NCH * h + kc, :],
                            start=(kc == 0),
                            stop=(kc == NCH - 1),
                            tile_position=(0, 32 * j),
                        )
            utb = []
            for bank in range(2):
                ub = win.tile([P, LW], BF16, tag="utb")
                nc.vector.tensor_copy(ub, uts[bank])
                utb.append(ub)
            # transpose U^T back to [qi, (head, d)], normalize, add pos, store
            for qc in range(NCH):
                tts = []
                for bank in range(2):
                    tt = scr_pool.tile([P, 512], BF16, tag="scr")
                    nc.tensor.transpose(
                        tt[:, 0:P], utb[bank][:, P * qc : P * (qc + 1)], identb
                    )
                    tts.append(tt)
                xn = xnp.tile([P, NH, HD], F32, tag="xn")
                for bank in range(2):
                    ttr = tts[bank][:, 0:P].rearrange("p (j x) -> p j x", x=32)
                    r4 = xnp.tile([P, 4], F32, tag="r4")
                    nc.vector.reciprocal(r4, ttr[:, :, HD : HD + 1])
                    nc.vector.tensor_mul(
                        xn[:, 4 * bank : 4 * bank + 4, :],
                        ttr[:, :, 0:HD],
                        r4[:, :, None].to_broadcast([P, 4, HD]),
                    )
                nc.vector.tensor_add(
                    xn, xn, post[:, w * NCH + qc, :].rearrange("p (n d) -> p n d", d=HD)
                )
                nc.sync.dma_start(
                    o3[16 * qc : 16 * qc + 16, SS * w : SS * w + SS, :],
                    xn.rearrange("p n d -> p (n d)"),
                )


def build_program():
    nc = bass.Bass("TRN2", target_bir_lowering=False)
    qw = nc.dram_tensor("qw", [BS, NWIN, LW, 2, P], BF16, kind="ExternalInput")
    kw = nc.dram_tensor("kw", [BS, NWIN, LW, 2, P], BF16, kind="ExternalInput")
    vaw = nc.dram_tensor("vaw", [BS, NWIN, LW, NH, 32], BF16, kind="ExternalInput")
    vimg = nc.dram_tensor(
        "vimg", [BS, P, H + 2, W + 2], BF16, kind="ExternalInput"
    )
    p1wT = nc.dram_tensor("p1wT", [C, C // 4], F32, kind="ExternalInput")
    eb1 = nc.dram_tensor("eb1", [C // 4, 1], F32, kind="ExternalInput")
    p2wT = nc.dram_tensor("p2wT", [C // 4, G * C], F32, kind="ExternalInput")
    p2br = nc.dram_tensor("p2br", [C, G], F32, kind="ExternalInput")
    dwb = nc.dram_tensor("dwb", [C, G, 10], F32, kind="ExternalInput")
    out = nc.dram_tensor("out", [BS, L, C], F32, kind="ExternalOutput")
    with tile.TileContext(nc) as tc:
        with ExitStack() as ctx:
            _emit(ctx, tc, qw[:], kw[:], vaw[:], vimg[:], p1wT[:], eb1[:],
                  p2wT[:], p2br[:], dwb[:], out[:])
    _split_sync_waits(nc)
    nc.finalize()
    return nc


def host_weights(inputs):
    """Host-side weight preprocessing (BN folding, transposes, layout)."""
    p1w = np.asarray(inputs["proj1_w"], np.float32)      # (32, 128)
    p1b = np.asarray(inputs["proj1_b"], np.float32)      # (32,)
    gam = np.asarray(inputs["bn_gamma"], np.float32)
    bet = np.asarray(inputs["bn_beta"], np.float32)
    rm = np.asarray(inputs["bn_mean"], np.float32)
    rv = np.asarray(inputs["bn_var"], np.float32)
    p2w = np.asarray(inputs["proj2_w"], np.float32)      # (512, 32)
    p2b = np.asarray(inputs["proj2_b"], np.float32)      # (512,)
    dyn_w = np.asarray(inputs["dyn_w"], np.float32)      # (4, 128, 3, 3)
    dyn_b = np.asarray(inputs["dyn_b"], np.float32)      # (4, 128)

    es = gam / np.sqrt(rv + 1e-5)                        # (32,)
    p1w_eff = p1w * es[:, None]
    eb1 = ((p1b - rm) * es + bet).reshape(C // 4, 1)
    p1wT = np.ascontiguousarray(p1w_eff.T)               # (128, 32)
    p2wT = np.ascontiguousarray((0.5 * p2w).T)           # (32, 512); 0.5 from gelu
    p2br = np.ascontiguousarray(p2b.reshape(G, C).T)     # (128, 4)
    dwr = np.ascontiguousarray(dyn_w.transpose(1, 0, 2, 3).reshape(C, G, 9))
    dbr = np.ascontiguousarray(dyn_b.T)                  # (128, 4)
    dwb = np.ascontiguousarray(
        np.concatenate([dwr, dbr[:, :, None]], axis=2)   # (128, 4, 10)
    )
    return dict(p1wT=p1wT, eb1=eb1, p2wT=p2wT, p2br=p2br, dwb=dwb)


def host_activations(q, k, v):
    """Window-major, per-head zero-padded bf16 layouts (pure layout/dtype)."""
    bf16 = ml_dtypes.bfloat16
    B = q.shape[0]
    # [b, l, c] -> [b, w, t, c] window-major (t = hs*SS + ws, l = hs*W + w*SS + ws)
    def win_major(x):
        x6 = x.reshape(B, H, NWIN, SS, C)                # (b, hs, w, ws, c)
        return np.ascontiguousarray(x6.transpose(0, 2, 1, 3, 4)).reshape(
            B, NWIN, LW, C
        )

    qm = win_major(q)
    km = win_major(k)
    vm = win_major(v)
    # per-head padded groups: [b, w, t, g, 32j + d] = x[..., 16*(4g+j)+d]
    def pad_groups(xm):
        out = np.zeros((B, NWIN, LW, 2, 4, 32), np.float32)
        xh = xm.reshape(B, NWIN, LW, 2, 4, HD)
        out[..., 0:HD] = xh
        return out.reshape(B, NWIN, LW, 2, P).astype(bf16)

    qw = pad_groups(qm)
    kw = pad_groups(km)
    # V window layout + ones column: [b, w, t, h, 32]
    vaw = np.zeros((B, NWIN, LW, NH, 32), np.float32)
    vaw[..., 0:HD] = vm.reshape(B, NWIN, LW, NH, HD)
    vaw[..., HD] = 1.0
    vaw = vaw.astype(bf16)
    # zero-padded conv image [b, c, H+2, W+2]
    vimg = np.zeros((B, C, H + 2, W + 2), np.float32)
    vimg[:, :, 1 : H + 1, 1 : W + 1] = v.transpose(0, 2, 1).reshape(B, C, H, W)
    vimg = vimg.astype(bf16)
    return dict(qw=qw, kw=kw, vaw=vaw, vimg=vimg)


_PROGRAM = None


def get_program():
    global _PROGRAM
    if _PROGRAM is None:
        _PROGRAM = build_program()
    return _PROGRAM


def make_in_maps(inputs):
    q = np.ascontiguousarray(np.asarray(inputs["q"], np.float32))
    k = np.ascontiguousarray(np.asarray(inputs["k"], np.float32))
    v = np.ascontiguousarray(np.asarray(inputs["v"], np.float32))
    wts = host_weights(inputs)
    acts = host_activations(q, k, v)
    in_maps = []
    for i in range(N_CORES):
        sl = slice(BS * i, BS * (i + 1))
        m = {name: arr[sl] for name, arr in acts.items()}
        m.update(wts)
        in_maps.append(m)
    return in_maps


def kernel(**inputs) -> np.ndarray:
    from concourse.bass_utils import run_bass_kernel_spmd

    nc = get_program()
    in_maps = make_in_maps(inputs)
    res = run_bass_kernel_spmd(
        nc, in_maps, list(range(N_CORES)),
        trace=bool(int(os.environ.get("KERNEL_TRACE", "0"))),
    )
    out = np.concatenate([res.results[i]["out"] for i in range(N_CORES)], axis=0)
    return np.ascontiguousarray(out.astype(np.float32))
